# revision 27
# baseline (speedup 1.0000x reference)
import sys
for p in ('/opt/trn_rl_repo', '/opt/pypackages'):
    if p not in sys.path:
        sys.path.insert(0, p)
import numpy as np
from concourse import bass, bacc, tile, mybir
from concourse import bass_utils

B, C, T, K = 4, 64, 4096, 4
NCORES = 8
OS = T // NCORES          # 512: per-core token slice (phase-1 o-slice == phase-2 t-slice)
BC = B * C                # 256
f32 = mybir.dt.float32
f16 = mybir.dt.float16
u32 = mybir.dt.uint32

_cache = {}
_last_exec_ns = []


def _build_l1():
    """Phase 1, SPMD core j: q/k/v projections for token slice j via fp16-split
    matmuls (one Ldweights per x tile serves all 7 uses), conv-folded u tables,
    then emit pre-split outputs: qn hi/lo f16, k hi/lo stacked f16."""
    nc = bacc.Bacc("TRN2", target_bir_lowering=False, debug=False, num_devices=NCORES)
    NKT = T // 128            # 32 contraction tiles
    HC = 4                    # kt per streaming chunk
    NCH = NKT // HC           # 4 chunks
    XH = nc.dram_tensor("xh", [128, NKT, BC], f16, kind="ExternalInput").ap()
    XL = nc.dram_tensor("xl", [128, NKT, BC], f16, kind="ExternalInput").ap()
    WQH = nc.dram_tensor("wqh", [128, NKT, OS], f16, kind="ExternalInput").ap()
    WQL = nc.dram_tensor("wql", [128, NKT, OS], f16, kind="ExternalInput").ap()
    WKH = nc.dram_tensor("wkh", [128, NKT, OS], f16, kind="ExternalInput").ap()
    WKL = nc.dram_tensor("wkl", [128, NKT, OS], f16, kind="ExternalInput").ap()
    WV = nc.dram_tensor("wv", [128, NKT, OS], f16, kind="ExternalInput").ap()
    CW = nc.dram_tensor("cw", [2 * C, K * C], f16, kind="ExternalInput").ap()
    QHO = nc.dram_tensor("qh_o", [B, C, OS], f16, kind="ExternalOutput").ap()
    QLO = nc.dram_tensor("ql_o", [B, C, OS], f16, kind="ExternalOutput").ap()
    KSO = nc.dram_tensor("kst_o", [B, 128, OS], f16, kind="ExternalOutput").ap()
    UO = nc.dram_tensor("u_o", [B, K, 4, 128, C], f16, kind="ExternalOutput").ap()

    with tile.TileContext(nc) as tc:
        with tc.tile_pool(name="xp", bufs=2) as xp, \
             tc.tile_pool(name="wp", bufs=2) as wp, \
             tc.tile_pool(name="sp", bufs=1) as sp, \
             tc.tile_pool(name="up", bufs=4) as up, \
             tc.tile_pool(name="tp", bufs=2) as tp, \
             tc.tile_pool(name="cp", bufs=1) as cp, \
             tc.tile_pool(name="ppq", bufs=1, space="PSUM") as ppq, \
             tc.tile_pool(name="ppk", bufs=1, space="PSUM") as ppk, \
             tc.tile_pool(name="ppv", bufs=1, space="PSUM") as ppv, \
             tc.tile_pool(name="pu", bufs=1, space="PSUM") as pu:
            cw = cp.tile([2 * C, K * C], f16, tag="cw")
            nc.sync.dma_start(out=cw[:, :], in_=CW[:, :])
            ones_r = cp.tile([128, 1], f32, tag="ones_r")   # reduce lhsT
            nc.vector.memset(ones_r[:, :], 1.0)
            ones_b = cp.tile([1, C], f32, tag="ones_b")     # broadcast lhsT
            nc.vector.memset(ones_b[:, :], 1.0)

            qacc = [ppq.tile([128, OS], f32, tag=f"q{mt}", name=f"qacc{mt}")
                    for mt in range(2)]
            kacc = [ppk.tile([128, OS], f32, tag=f"k{mt}", name=f"kacc{mt}")
                    for mt in range(2)]
            vacc = [ppv.tile([128, OS], f32, tag=f"v{mt}", name=f"vacc{mt}")
                    for mt in range(2)]

            for ch in range(NCH):
                ksl = slice(ch * HC, (ch + 1) * HC)
                xh = xp.tile([128, HC, BC], f16, tag="xh")
                xl = xp.tile([128, HC, BC], f16, tag="xl")
                nc.sync.dma_start(out=xh[:, :, :], in_=XH[:, ksl, :])
                nc.sync.dma_start(out=xl[:, :, :], in_=XL[:, ksl, :])
                wqh = wp.tile([128, HC, OS], f16, tag="wqh")
                wql = wp.tile([128, HC, OS], f16, tag="wql")
                wkh = wp.tile([128, HC, OS], f16, tag="wkh")
                wkl = wp.tile([128, HC, OS], f16, tag="wkl")
                wv = wp.tile([128, HC, OS], f16, tag="wv")
                nc.scalar.dma_start(out=wqh[:, :, :], in_=WQH[:, ksl, :])
                nc.scalar.dma_start(out=wkh[:, :, :], in_=WKH[:, ksl, :])
                nc.scalar.dma_start(out=wql[:, :, :], in_=WQL[:, ksl, :])
                nc.scalar.dma_start(out=wkl[:, :, :], in_=WKL[:, ksl, :])
                nc.gpsimd.dma_start(out=wv[:, :, :], in_=WV[:, ksl, :])
                for t in range(HC):
                    kt = ch * HC + t
                    first = (kt == 0)
                    last = (kt == NKT - 1)
                    for mt in range(2):
                        lh = xh[:, t, mt * 128:(mt + 1) * 128]
                        ll = xl[:, t, mt * 128:(mt + 1) * 128]
                        # one Ldweights of lh serves 5 matmuls, ll serves 2
                        nc.tensor.matmul(out=qacc[mt][:, :], lhsT=lh, rhs=wqh[:, t, :],
                                         start=first, stop=False)
                        nc.tensor.matmul(out=kacc[mt][:, :], lhsT=lh, rhs=wkh[:, t, :],
                                         start=first, stop=False)
                        nc.tensor.matmul(out=vacc[mt][:, :], lhsT=lh, rhs=wv[:, t, :],
                                         start=first, stop=last)
                        nc.tensor.matmul(out=qacc[mt][:, :], lhsT=lh, rhs=wql[:, t, :],
                                         start=False, stop=False)
                        nc.tensor.matmul(out=kacc[mt][:, :], lhsT=lh, rhs=wkl[:, t, :],
                                         start=False, stop=False)
                        nc.tensor.matmul(out=qacc[mt][:, :], lhsT=ll, rhs=wqh[:, t, :],
                                         start=False, stop=last)
                        nc.tensor.matmul(out=kacc[mt][:, :], lhsT=ll, rhs=wkh[:, t, :],
                                         start=False, stop=last)

            # ---- V to f16, then u tables: u[b,k] = (v_b^T @ cw_k) ----
            vsb = {}
            for mt in range(2):
                v16 = sp.tile([128, OS], f16, tag=f"vsb{mt}")
                nc.scalar.copy(out=v16[:, :], in_=vacc[mt][:, :])
                vsb[mt] = v16
            pub = pu.tile([1, OS], f32, tag="pub")
            # ---- k: split into f16 hi/lo, write stacked [hi;lo] ----
            for mt in range(2):
                ks = tp.tile([128, OS], f32, tag=f"ksb{mt}")
                nc.scalar.copy(out=ks[:, :], in_=kacc[mt][:, :])
                kh = tp.tile([128, OS], f16, tag="kh")
                nc.scalar.copy(out=kh[:, :], in_=ks[:, :])
                kh32 = tp.tile([128, OS], f32, tag="kh32")
                nc.scalar.copy(out=kh32[:, :], in_=kh[:, :])
                kl = tp.tile([128, OS], f16, tag="kl")
                nc.vector.tensor_sub(out=kl[:, :], in0=ks[:, :], in1=kh32[:, :])
                for half in range(2):
                    b = mt * 2 + half
                    off = half * C
                    nc.sync.dma_start(out=KSO[b, 0:C, :], in_=kh[off:off + C, :])
                    nc.sync.dma_start(out=KSO[b, C:2 * C, :], in_=kl[off:off + C, :])

            # ---- qn = q / ||q||_col, then split hi/lo f16 ----
            qs, sq, qn = {}, {}, {}
            for mt in range(2):
                qs[mt] = tp.tile([128, OS], f32, tag=f"qsb{mt}", name=f"qs{mt}")
                nc.scalar.copy(out=qs[mt][:, :], in_=qacc[mt][:, :])
                sq[mt] = tp.tile([128, OS], f32, tag=f"sq{mt}", name=f"sq{mt}")
                nc.scalar.square(out=sq[mt][:, :], in_=qs[mt][:, :])
                qn[mt] = tp.tile([128, OS], f32, tag=f"qn{mt}", name=f"qn{mt}")
            nrm, rcp = {}, {}
            for b in range(B):
                mt, off = b // 2, (b % 2) * C
                nc.tensor.matmul(out=pub[:, :], lhsT=ones_r[off:off + C, :],
                                 rhs=sq[mt][off:off + C, :], start=True, stop=True)
                nrm[b] = sp.tile([1, OS], f32, tag=f"nrm{b}", name=f"nrm{b}")
                nc.scalar.sqrt(out=nrm[b][:, :], in_=pub[:, :])
            for b in range(B):
                rcp[b] = sp.tile([1, OS], f32, tag=f"rcp{b}", name=f"rcp{b}")
                nc.vector.reciprocal(out=rcp[b][:, :], in_=nrm[b][:, :])
            for b in range(B):
                mt, off = b // 2, (b % 2) * C
                # freed K accumulators host the broadcast
                nc.tensor.matmul(out=kacc[mt][off:off + C, :], lhsT=ones_b[:, :],
                                 rhs=rcp[b][:, :], start=True, stop=True)
            for mt in range(2):
                bc = tp.tile([128, OS], f32, tag=f"bc{mt}", name=f"bc{mt}")
                nc.scalar.copy(out=bc[:, :], in_=kacc[mt][:, :])
                nc.vector.tensor_mul(out=qn[mt][:, :], in0=qs[mt][:, :],
                                     in1=bc[:, :])
            for mt in range(2):
                qh = tp.tile([128, OS], f16, tag="qh")
                nc.scalar.copy(out=qh[:, :], in_=qn[mt][:, :])
                qh32 = tp.tile([128, OS], f32, tag="qh32")
                nc.scalar.copy(out=qh32[:, :], in_=qh[:, :])
                qlo = tp.tile([128, OS], f16, tag="qlo")
                nc.vector.tensor_sub(out=qlo[:, :], in0=qn[mt][:, :], in1=qh32[:, :])
                for half in range(2):
                    b = mt * 2 + half
                    off = half * C
                    nc.sync.dma_start(out=QHO[b], in_=qh[off:off + C, :])
                    nc.sync.dma_start(out=QLO[b], in_=qlo[off:off + C, :])
            for b in range(B):
                off = (b % 2) * C
                vt = vsb[b // 2][off:off + C, :]  # [64, 512] f16
                for k in range(K):
                    # freed V accumulators serve as pingpong psum for u tiles
                    pacc = vacc[(b * K + k) % 2]
                    u16 = up.tile([128, 4, C], f16, tag="u16")
                    for m in range(4):
                        nc.tensor.matmul(out=pacc[:, m * C:(m + 1) * C],
                                         lhsT=vt[:, m * 128:(m + 1) * 128],
                                         rhs=cw[off:off + C, k * C:(k + 1) * C],
                                         start=True, stop=True)
                    nc.scalar.copy(out=u16[:, :, :], in_=pacc[:, 0:4 * C])
                    nc.scalar.dma_start(
                        out=UO[b, k].rearrange("m p c -> p m c")[:, :, :],
                        in_=u16[:, :, :])

    nc.compile()
    return nc


def _build_l2():
    """Phase 2, SPMD core j: rows t in [j*512,(j+1)*512) for all batches.
    sim = k_t . qn_s via stacked [hi;lo] 2-matmul fp16 split, exact top-4 via
    max/max_index, gather-sum of u tables. Batch-major tile order so each
    batch's out matmul spreads through the run; batch 3's accumulates
    cross-phase in persistent psum so only the stop-chunk lands in the tail."""
    nc = bacc.Bacc("TRN2", target_bir_lowering=False, debug=False, num_devices=NCORES)
    QH = nc.dram_tensor("qh", [B, 128, T], f16, kind="ExternalInput").ap()
    QL = nc.dram_tensor("ql", [B, C, T], f16, kind="ExternalInput").ap()
    KST = nc.dram_tensor("kst", [128, B * OS], f16, kind="ExternalInput").ap()
    WOT = nc.dram_tensor("wot", [OS, T], f16, kind="ExternalInput").ap()
    UT = [[nc.dram_tensor(f"ut{b}_{k}", [T, C], f16, kind="ExternalInput").ap()
           for k in range(K)] for b in range(B)]
    OUT = nc.dram_tensor("out_o", [2, 128, T], f32, kind="ExternalOutput").ap()

    with tile.TileContext(nc) as tc:
        with tc.tile_pool(name="qp", bufs=1) as qp, \
             tc.tile_pool(name="wp", bufs=1) as wp, \
             tc.tile_pool(name="sp", bufs=2) as sp, \
             tc.tile_pool(name="simp", bufs=3) as simp, \
             tc.tile_pool(name="yp", bufs=1) as yp, \
             tc.tile_pool(name="op", bufs=2) as op, \
             tc.tile_pool(name="pp", bufs=2, space="PSUM") as pp, \
             tc.tile_pool(name="po", bufs=1, space="PSUM") as po:
            kst_flat = qp.tile([128, B * OS], f16, tag="kst")
            kst = kst_flat.rearrange("p (b o) -> p b o", b=B)
            qhd, qlt = {}, {}
            for b in range(B):
                qhd[b] = qp.tile([128, T], f16, tag=f"qhd{b}", name=f"qhd{b}")
                qlt[b] = qp.tile([C, T], f16, tag=f"qlt{b}", name=f"qlt{b}")
            # all loads on the SP queue in need-order: ACT must stay clear for
            # sim copies, gpsimd for gathers
            nc.sync.dma_start(out=kst_flat[:, :], in_=KST[:, :])
            nc.sync.dma_start(out=qhd[0][:, :], in_=QH[0])
            nc.sync.dma_start(out=qlt[0][:, :], in_=QL[0])
            wot = {}
            for kt in range(4):
                wot[kt] = wp.tile([128, T], f16, tag=f"wot{kt}", name=f"wot{kt}")
            for b in range(1, B):
                nc.sync.dma_start(out=qhd[b][:, :], in_=QH[b])
                nc.sync.dma_start(out=qlt[b][:, :], in_=QL[b])
            for kt in range(4):
                nc.sync.dma_start(out=wot[kt][:, :],
                                  in_=WOT[kt * 128:(kt + 1) * 128, :])

            _pob3t = [po.tile([128, 1024], f32, tag=f"po3_{j}", name=f"pob3_{j}")
                      for j in range(2)]
            pob3 = [_pob3t[c2 // 2][(c2 % 2) * C:(c2 % 2) * C + C, :]
                    for c2 in range(4)]
            ytp = {}  # (b, kt) -> [128, C] f16 lhsT tiles for the out matmul
            for b in range(B):
                for kt in range(4):
                    ytp[(b, kt)] = yp.tile([128, C], f16, tag=f"yt{b}{kt}",
                                           name=f"ytp{b}{kt}")

            def emit_out_chunk(bo, c2):
                # one 1024-col chunk of batch bo's out; psum from shared pool
                boff = (bo % 2) * C
                pob = pp.tile([128, 1024], f32, tag="ps", name="pob")[0:C, :]
                for kt in range(4):
                    for q in range(2):
                        ch = c2 * 2 + q
                        nc.tensor.matmul(
                            out=pob[:, q * 512:(q + 1) * 512],
                            lhsT=ytp[(bo, kt)][:, :],
                            rhs=wot[kt][:, ch * 512:(ch + 1) * 512],
                            start=(kt == 0), stop=(kt == 3))
                ob = op.tile([C, 1024], f32, tag="ob")
                nc.scalar.copy(out=ob[:, :], in_=pob[:, :])
                nc.sync.dma_start(
                    out=OUT[bo // 2, boff:boff + C, c2 * 1024:(c2 + 1) * 1024],
                    in_=ob[:, :])

            for b in range(B):      # batch-major: out(b) unblocks after 4 tiles
                for i in range(4):  # 128-token blocks of this core's slice
                    ksl = kst[:, b, i * 128:(i + 1) * 128]       # [128,128] hi;lo
                    ksh = kst[0:C, b, i * 128:(i + 1) * 128]     # [64,128] hi only
                    sim = simp.tile([128, T], f32, tag="sim")
                    for c2 in range(4):
                        ps = pp.tile([128, 1024], f32, tag="ps")
                        for q in range(2):
                            ch = c2 * 2 + q
                            rh = qhd[b][:, ch * 512:(ch + 1) * 512]
                            rl = qlt[b][:, ch * 512:(ch + 1) * 512]
                            po_s = ps[:, q * 512:(q + 1) * 512]
                            # hi.hi + lo.hi (stacked contraction 128), then hi.lo
                            nc.tensor.matmul(out=po_s, lhsT=ksl, rhs=rh,
                                             start=True, stop=False)
                            nc.tensor.matmul(out=po_s, lhsT=ksh, rhs=rl,
                                             start=False, stop=True)
                        nc.scalar.copy(out=sim[:, c2 * 1024:(c2 + 1) * 1024],
                                       in_=ps[:, :])

                    # previous batch's out chunk c2=i runs here: after this
                    # tile's sim matmuls, overlapping its topk
                    if b > 0:
                        emit_out_chunk(b - 1, i)
                    if b == 3 and i > 0:
                        # batch 3: accumulate kt-block i-1 into persistent psum
                        for c2 in range(4):
                            for q in range(2):
                                ch = c2 * 2 + q
                                nc.tensor.matmul(
                                    out=pob3[c2][:, q * 512:(q + 1) * 512],
                                    lhsT=ytp[(3, i - 1)][:, :],
                                    rhs=wot[i - 1][:, ch * 512:(ch + 1) * 512],
                                    start=(i == 1), stop=False)

                    m8 = sp.tile([128, 8], f32, tag="m8")
                    i8 = sp.tile([128, 8], u32, tag="i8")
                    nc.vector.max(out=m8[:, :], in_=sim[:, :])
                    nc.vector.max_index(out=i8[:, :], in_max=m8[:, :], in_values=sim[:, :])
                    gth = sp.tile([128, K, C], f16, tag="gth")
                    for k in range(K):
                        nc.gpsimd.indirect_dma_start(
                            out=gth[:, k, :], out_offset=None,
                            in_=UT[b][k][:, :],
                            in_offset=bass.IndirectOffsetOnAxis(ap=i8[:, k:k + 1], axis=0))
                    t0 = sp.tile([128, C], f16, tag="t0")
                    t1 = sp.tile([128, C], f16, tag="t1")
                    nc.gpsimd.tensor_add(out=t0[:, :], in0=gth[:, 0, :], in1=gth[:, 1, :])
                    nc.gpsimd.tensor_add(out=t1[:, :], in0=gth[:, 2, :], in1=gth[:, 3, :])
                    nc.gpsimd.tensor_add(out=ytp[(b, i)][:, :], in0=t0[:, :],
                                         in1=t1[:, :])

            # tail: batch 3's final kt block + copies + stores
            for c2 in range(4):
                for q in range(2):
                    ch = c2 * 2 + q
                    nc.tensor.matmul(
                        out=pob3[c2][:, q * 512:(q + 1) * 512],
                        lhsT=ytp[(3, 3)][:, :],
                        rhs=wot[3][:, ch * 512:(ch + 1) * 512],
                        start=False, stop=True)
                ob = op.tile([C, 1024], f32, tag="ob")
                nc.scalar.copy(out=ob[:, :], in_=pob3[c2][:, :])
                nc.sync.dma_start(
                    out=OUT[1, C:2 * C, c2 * 1024:(c2 + 1) * 1024],
                    in_=ob[:, :])
    nc.compile()
    return nc


def _split16(a):
    h = a.astype(np.float16)
    l = (a - h.astype(np.float32)).astype(np.float16)
    return h, l


def kernel(x, Wq, Wk, Wv, Wo, conv_w, conv_b):
    x = np.asarray(x, np.float32)
    Wq = np.asarray(Wq, np.float32); Wk = np.asarray(Wk, np.float32)
    Wv = np.asarray(Wv, np.float32); Wo = np.asarray(Wo, np.float32)
    conv_w = np.asarray(conv_w, np.float32); conv_b = np.asarray(conv_b, np.float32)

    if "l1" not in _cache:
        _cache["l1"] = _build_l1()
    if "l2" not in _cache:
        _cache["l2"] = _build_l2()

    def _sw(a):
        # [T, W] -> [128, T//128, W] with [p, kt, w] = a[kt*128+p, w]
        return np.ascontiguousarray(a.reshape(T // 128, 128, -1).transpose(1, 0, 2))

    xT = np.ascontiguousarray(x.transpose(2, 0, 1).reshape(T, BC))  # [t, b*64+c]
    xh, xl = _split16(xT)
    xh, xl = _sw(xh), _sw(xl)
    WqT, WkT = Wq.T, Wk.T
    WvT16 = np.ascontiguousarray(Wv.T).astype(np.float16)
    cw1 = np.ascontiguousarray(conv_w.transpose(1, 2, 0).reshape(C, K * C)).astype(np.float16)
    cw = np.concatenate([cw1, cw1], axis=0)
    # cw[ci, k*64+co] = conv_w[co, ci, k]

    in_maps = []
    for j in range(NCORES):
        sl = slice(j * OS, (j + 1) * OS)
        wqh, wql = _split16(np.ascontiguousarray(WqT[:, sl]))
        wkh, wkl = _split16(np.ascontiguousarray(WkT[:, sl]))
        in_maps.append({"xh": xh, "xl": xl,
                        "wqh": _sw(wqh), "wql": _sw(wql),
                        "wkh": _sw(wkh), "wkl": _sw(wkl),
                        "wv": _sw(np.ascontiguousarray(WvT16[:, sl])), "cw": cw})
    global _last_exec_ns
    _last_exec_ns = []
    r1 = bass_utils.run_bass_kernel_spmd(_cache["l1"], in_maps, core_ids=list(range(NCORES)))
    _last_exec_ns.append(r1.exec_time_ns)

    qh1 = np.concatenate([r1.results[j]["qh_o"] for j in range(NCORES)], axis=2)  # [B,C,T]
    qh = np.ascontiguousarray(np.concatenate([qh1, qh1], axis=1))  # [B,128,T] dup
    ql = np.ascontiguousarray(np.concatenate(
        [r1.results[j]["ql_o"] for j in range(NCORES)], axis=2))
    ut = {}
    for b in range(B):
        for k in range(K):
            ut[(b, k)] = np.ascontiguousarray(np.concatenate(
                [r1.results[j]["u_o"][b, k].reshape(OS, C) for j in range(NCORES)], axis=0))

    in_maps2 = []
    for j in range(NCORES):
        kst_j = np.ascontiguousarray(
            r1.results[j]["kst_o"].transpose(1, 0, 2).reshape(128, B * OS))
        m = {"qh": qh, "ql": ql, "kst": kst_j,
             "wot": np.ascontiguousarray(Wo.T[j * OS:(j + 1) * OS, :]).astype(np.float16)}
        for b in range(B):
            for k in range(K):
                m[f"ut{b}_{k}"] = ut[(b, k)]
        in_maps2.append(m)
    r2 = bass_utils.run_bass_kernel_spmd(_cache["l2"], in_maps2, core_ids=list(range(NCORES)))
    _last_exec_ns.append(r2.exec_time_ns)

    out = np.zeros((B, C, T), np.float32)
    for j in range(NCORES):
        oo = r2.results[j]["out_o"]  # [2, 128, T]
        for b in range(B):
            out[b] += oo[b // 2, (b % 2) * C:(b % 2) * C + C, :]
    bias = conv_b[:, None] * Wo.sum(axis=1)[None, :]  # [64, 4096]
    out += bias[None, :, :]
    return out


# revision 34
# speedup vs baseline: 1.0537x; 1.0537x over previous
import sys
for p in ('/opt/trn_rl_repo', '/opt/pypackages'):
    if p not in sys.path:
        sys.path.insert(0, p)
import numpy as np
from concourse import bass, bacc, tile, mybir
from concourse import bass_utils

B, C, T, K = 4, 64, 4096, 4
NCORES = 8
OS = T // NCORES          # 512: per-core token slice (phase-1 o-slice == phase-2 t-slice)
BC = B * C                # 256
f32 = mybir.dt.float32
f16 = mybir.dt.float16
u32 = mybir.dt.uint32

_cache = {}
_last_exec_ns = []


def _build_l1():
    """Phase 1, SPMD core j: q/k/v projections for token slice j via fp16-split
    matmuls (one Ldweights per x tile serves all 7 uses), conv-folded u tables,
    then emit pre-split outputs: qn hi/lo f16, k hi/lo stacked f16."""
    nc = bacc.Bacc("TRN2", target_bir_lowering=False, debug=False, num_devices=NCORES)
    NKT = T // 128            # 32 contraction tiles
    HC = 4                    # kt per streaming chunk
    NCH = NKT // HC           # 4 chunks
    XH = nc.dram_tensor("xh", [128, NKT, BC], f16, kind="ExternalInput").ap()
    XL = nc.dram_tensor("xl", [128, NKT, BC], f16, kind="ExternalInput").ap()
    WQH = nc.dram_tensor("wqh", [128, NKT, OS], f16, kind="ExternalInput").ap()
    WQL = nc.dram_tensor("wql", [128, NKT, OS], f16, kind="ExternalInput").ap()
    WKH = nc.dram_tensor("wkh", [128, NKT, OS], f16, kind="ExternalInput").ap()
    WKL = nc.dram_tensor("wkl", [128, NKT, OS], f16, kind="ExternalInput").ap()
    WV = nc.dram_tensor("wv", [128, NKT, OS], f16, kind="ExternalInput").ap()
    CW = nc.dram_tensor("cw", [2 * C, K * C], f16, kind="ExternalInput").ap()
    QHO = nc.dram_tensor("qh_o", [B, C, OS], f16, kind="ExternalOutput").ap()
    QLO = nc.dram_tensor("ql_o", [B, C, OS], f16, kind="ExternalOutput").ap()
    KSH = nc.dram_tensor("ksh_o", [B, C, OS], f16, kind="ExternalOutput").ap()
    KSL = nc.dram_tensor("ksl_o", [B, C, OS], f16, kind="ExternalOutput").ap()
    UO = nc.dram_tensor("u_o", [B, K, 4, 128, C], f16, kind="ExternalOutput").ap()

    with tile.TileContext(nc) as tc:
        with tc.tile_pool(name="xp", bufs=2) as xp, \
             tc.tile_pool(name="wp", bufs=2) as wp, \
             tc.tile_pool(name="sp", bufs=1) as sp, \
             tc.tile_pool(name="up", bufs=4) as up, \
             tc.tile_pool(name="tp", bufs=2) as tp, \
             tc.tile_pool(name="cp", bufs=1) as cp, \
             tc.tile_pool(name="ppq", bufs=1, space="PSUM") as ppq, \
             tc.tile_pool(name="ppk", bufs=1, space="PSUM") as ppk, \
             tc.tile_pool(name="ppv", bufs=1, space="PSUM") as ppv, \
             tc.tile_pool(name="pu", bufs=1, space="PSUM") as pu:
            cw = cp.tile([2 * C, K * C], f16, tag="cw")
            nc.sync.dma_start(out=cw[:, :], in_=CW[:, :])
            ones_r = cp.tile([128, 1], f32, tag="ones_r")   # reduce lhsT
            nc.vector.memset(ones_r[:, :], 1.0)
            ones_b = cp.tile([1, C], f32, tag="ones_b")     # broadcast lhsT
            nc.vector.memset(ones_b[:, :], 1.0)

            gate = cp.tile([1, 1], f16, tag="gate")
            qacc = [ppq.tile([128, OS], f32, tag=f"q{mt}", name=f"qacc{mt}")
                    for mt in range(2)]
            kacc = [ppk.tile([128, OS], f32, tag=f"k{mt}", name=f"kacc{mt}")
                    for mt in range(2)]
            vacc = [ppv.tile([128, OS], f32, tag=f"v{mt}", name=f"vacc{mt}")
                    for mt in range(2)]

            for ch in range(NCH):
                ksl = slice(ch * HC, (ch + 1) * HC)
                xh = xp.tile([128, HC, BC], f16, tag="xh")
                xl = xp.tile([128, HC, BC], f16, tag="xl")
                nc.sync.dma_start(out=xh[:, :, :], in_=XH[:, ksl, :])
                nc.sync.dma_start(out=xl[:, :, :], in_=XL[:, ksl, :])
                wqh = wp.tile([128, HC, OS], f16, tag="wqh")
                wql = wp.tile([128, HC, OS], f16, tag="wql")
                wkh = wp.tile([128, HC, OS], f16, tag="wkh")
                wkl = wp.tile([128, HC, OS], f16, tag="wkl")
                wv = wp.tile([128, HC, OS], f16, tag="wv")
                nc.scalar.dma_start(out=wqh[:, :, :], in_=WQH[:, ksl, :])
                nc.gpsimd.dma_start(out=wkh[:, :, :], in_=WKH[:, ksl, :])
                nc.scalar.dma_start(out=wql[:, :, :], in_=WQL[:, ksl, :])
                nc.gpsimd.dma_start(out=wkl[:, :, :], in_=WKL[:, ksl, :])
                nc.gpsimd.dma_start(out=wv[:, :, :], in_=WV[:, ksl, :])
                for t in range(HC):
                    kt = ch * HC + t
                    first = (kt == 0)
                    last = (kt == NKT - 1)
                    for mt in range(2):
                        lh = xh[:, t, mt * 128:(mt + 1) * 128]
                        ll = xl[:, t, mt * 128:(mt + 1) * 128]
                        # one Ldweights of lh serves 5 matmuls, ll serves 2
                        nc.tensor.matmul(out=qacc[mt][:, :], lhsT=lh, rhs=wqh[:, t, :],
                                         start=first, stop=False)
                        nc.tensor.matmul(out=kacc[mt][:, :], lhsT=lh, rhs=wkh[:, t, :],
                                         start=first, stop=False)
                        nc.tensor.matmul(out=vacc[mt][:, :], lhsT=lh, rhs=wv[:, t, :],
                                         start=first, stop=last)
                        nc.tensor.matmul(out=qacc[mt][:, :], lhsT=lh, rhs=wql[:, t, :],
                                         start=False, stop=False)
                        nc.tensor.matmul(out=kacc[mt][:, :], lhsT=lh, rhs=wkl[:, t, :],
                                         start=False, stop=False)
                        nc.tensor.matmul(out=qacc[mt][:, :], lhsT=ll, rhs=wqh[:, t, :],
                                         start=False, stop=last)
                        nc.tensor.matmul(out=kacc[mt][:, :], lhsT=ll, rhs=wkh[:, t, :],
                                         start=False, stop=last)

            # ---- V to f16 ----
            vsb = {}
            for mt in range(2):
                v16 = sp.tile([128, OS], f16, tag=f"vsb{mt}")
                nc.scalar.copy(out=v16[:, :], in_=vacc[mt][:, :])
                vsb[mt] = v16
            pub = [pu.tile([1, OS], f32, tag=f"pub{j}", name=f"pub{j}")
                   for j in range(2)]
            # ---- k: split into f16 hi/lo ----
            for mt in range(2):
                ks = tp.tile([128, OS], f32, tag=f"ksb{mt}")
                nc.scalar.copy(out=ks[:, :], in_=kacc[mt][:, :])
                kh = tp.tile([128, OS], f16, tag="kh")
                nc.scalar.copy(out=kh[:, :], in_=ks[:, :])
                kl = tp.tile([128, OS], f16, tag="kl")
                nc.vector.tensor_sub(out=kl[:, :], in0=ks[:, :], in1=kh[:, :])
                ksh2 = KSH[2 * mt:2 * mt + 2].rearrange("b c o -> (b c) o")
                ksl2 = KSL[2 * mt:2 * mt + 2].rearrange("b c o -> (b c) o")
                nc.sync.dma_start(out=ksh2[:, :], in_=kh[:, :])
                nc.sync.dma_start(out=ksl2[:, :], in_=kl[:, :])
            # ---- qn norms: squares + row-sums + sqrt + recip, pipelined ----
            qs, sq, qn = {}, {}, {}
            for mt in range(2):
                qs[mt] = tp.tile([128, OS], f32, tag=f"qsb{mt}", name=f"qs{mt}")
                nc.scalar.copy(out=qs[mt][:, :], in_=qacc[mt][:, :])
                sq[mt] = tp.tile([128, OS], f32, tag=f"sq{mt}", name=f"sq{mt}")
                nc.scalar.square(out=sq[mt][:, :], in_=qs[mt][:, :])
                qn[mt] = tp.tile([128, OS], f32, tag=f"qn{mt}", name=f"qn{mt}")
            nrm, rcp = {}, {}
            for b in range(B):
                mt, off = b // 2, (b % 2) * C
                nc.tensor.matmul(out=pub[b % 2][:, :], lhsT=ones_r[off:off + C, :],
                                 rhs=sq[mt][off:off + C, :], start=True, stop=True)
                nrm[b] = sp.tile([1, OS], f32, tag=f"nrm{b}", name=f"nrm{b}")
                nc.scalar.sqrt(out=nrm[b][:, :], in_=pub[b % 2][:, :])
            for b in range(B):
                rcp[b] = sp.tile([1, OS], f32, tag=f"rcp{b}", name=f"rcp{b}")
                nc.vector.reciprocal(out=rcp[b][:, :], in_=nrm[b][:, :])
            # ---- u tables (PE + ACT copies interleave with the norm tail) ----
            for b in range(B):
                off = (b % 2) * C
                vt = vsb[b // 2][off:off + C, :]  # [64, 512] f16
                ust = up.tile([128, K, 4, C], f16, tag="ust")
                for k in range(K):
                    pacc = vacc[k % 2]
                    for m in range(4):
                        nc.tensor.matmul(out=pacc[:, m * C:(m + 1) * C],
                                         lhsT=vt[:, m * 128:(m + 1) * 128],
                                         rhs=cw[off:off + C, k * C:(k + 1) * C],
                                         start=True, stop=True)
                    nc.scalar.copy(out=ust[:, k, :, :], in_=pacc[:, 0:4 * C])
                nc.sync.dma_start(
                    out=UO[b].rearrange("k m p c -> p k m c")[:, :, :, :],
                    in_=ust[:, :, :, :])
            # ---- qn broadcast-scale + split hi/lo ----
            for b in range(B):
                mt, off = b // 2, (b % 2) * C
                nc.tensor.matmul(out=kacc[mt][off:off + C, :], lhsT=ones_b[:, :],
                                 rhs=rcp[b][:, :], start=True, stop=True)
            for mt in range(2):
                bc = tp.tile([128, OS], f32, tag=f"bc{mt}", name=f"bc{mt}")
                nc.scalar.copy(out=bc[:, :], in_=kacc[mt][:, :])
                nc.vector.tensor_mul(out=qn[mt][:, :], in0=qs[mt][:, :],
                                     in1=bc[:, :])
            for mt in range(2):
                qh = tp.tile([128, OS], f16, tag="qh")
                nc.scalar.copy(out=qh[:, :], in_=qn[mt][:, :])
                qlo = tp.tile([128, OS], f16, tag="qlo")
                nc.vector.tensor_sub(out=qlo[:, :], in0=qn[mt][:, :], in1=qh[:, :])
                qho2 = QHO[2 * mt:2 * mt + 2].rearrange("b c o -> (b c) o")
                qlo2 = QLO[2 * mt:2 * mt + 2].rearrange("b c o -> (b c) o")
                nc.sync.dma_start(out=qho2[:, :], in_=qh[:, :])
                nc.sync.dma_start(out=qlo2[:, :], in_=qlo[:, :])
    nc.compile()
    return nc


def _build_l2():
    """Phase 2, SPMD core j: rows t in [j*512,(j+1)*512) for all batches.
    sim = k_t . qn_s via stacked [hi;lo] 2-matmul fp16 split, exact top-4 via
    max/max_index, gather-sum of u tables. Batch-major tile order so each
    batch's out matmul spreads through the run; batch 3's accumulates
    cross-phase in persistent psum so only the stop-chunk lands in the tail."""
    nc = bacc.Bacc("TRN2", target_bir_lowering=False, debug=False, num_devices=NCORES)
    QH = nc.dram_tensor("qh", [B, 128, T], f16, kind="ExternalInput").ap()
    QL = nc.dram_tensor("ql", [B, C, T], f16, kind="ExternalInput").ap()
    KST = nc.dram_tensor("kst", [128, B * OS], f16, kind="ExternalInput").ap()
    WOT = nc.dram_tensor("wot", [OS, T], f16, kind="ExternalInput").ap()
    UT = [[nc.dram_tensor(f"ut{b}_{k}", [T, C], f16, kind="ExternalInput").ap()
           for k in range(K)] for b in range(B)]
    OUT = nc.dram_tensor("out_o", [2, 128, T], f32, kind="ExternalOutput").ap()

    with tile.TileContext(nc) as tc:
        with tc.tile_pool(name="qp", bufs=1) as qp, \
             tc.tile_pool(name="wp", bufs=1) as wp, \
             tc.tile_pool(name="sp", bufs=2) as sp, \
             tc.tile_pool(name="simp", bufs=3) as simp, \
             tc.tile_pool(name="yp", bufs=1) as yp, \
             tc.tile_pool(name="op", bufs=2) as op, \
             tc.tile_pool(name="pp", bufs=2, space="PSUM") as pp, \
             tc.tile_pool(name="po", bufs=1, space="PSUM") as po:
            kst_flat = qp.tile([128, B * OS], f16, tag="kst")
            kst = kst_flat.rearrange("p (b o) -> p b o", b=B)
            qhd, qlt = {}, {}
            for b in range(B):
                qhd[b] = qp.tile([128, T], f16, tag=f"qhd{b}", name=f"qhd{b}")
                qlt[b] = qp.tile([C, T], f16, tag=f"qlt{b}", name=f"qlt{b}")
            # all loads on the SP queue in need-order: ACT must stay clear for
            # sim copies, gpsimd for gathers
            nc.scalar.dma_start(out=kst_flat[:, :], in_=KST[:, :])
            nc.sync.dma_start(out=qhd[0][:, 0:2048], in_=QH[0][:, 0:2048])
            nc.sync.dma_start(out=qlt[0][:, 0:2048], in_=QL[0][:, 0:2048])
            nc.sync.dma_start(out=qhd[0][:, 2048:T], in_=QH[0][:, 2048:T])
            nc.sync.dma_start(out=qlt[0][:, 2048:T], in_=QL[0][:, 2048:T])
            wot = {}
            for kt in range(4):
                wot[kt] = wp.tile([128, T], f16, tag=f"wot{kt}", name=f"wot{kt}")
            for b in range(1, B):
                nc.sync.dma_start(out=qhd[b][:, :], in_=QH[b])
                nc.sync.dma_start(out=qlt[b][:, :], in_=QL[b])
            for kt in range(4):
                nc.sync.dma_start(out=wot[kt][:, :],
                                  in_=WOT[kt * 128:(kt + 1) * 128, :])

            ytp = {}  # (b, kt) -> [128, C] f16 lhsT tiles for the out matmul
            for b in range(B):
                for kt in range(4):
                    ytp[(b, kt)] = yp.tile([128, C], f16, tag=f"yt{b}{kt}",
                                           name=f"ytp{b}{kt}")
            pot = [po.tile([128, 1024], f32, tag=f"pot{j}", name=f"pot{j}")
                   for j in range(2)]
            pob3 = [pot[c2 // 2][(c2 % 2) * C:(c2 % 2) * C + C, :]
                    for c2 in range(4)]

            def emit_out_chunk(bo, c2):
                # one 1024-col chunk of batch bo's out; psum from shared pool
                boff = (bo % 2) * C
                pob = pp.tile([128, 1024], f32, tag="ps", name="pob")[0:C, :]
                for kt in range(4):
                    for q in range(2):
                        ch = c2 * 2 + q
                        nc.tensor.matmul(
                            out=pob[:, q * 512:(q + 1) * 512],
                            lhsT=ytp[(bo, kt)][:, :],
                            rhs=wot[kt][:, ch * 512:(ch + 1) * 512],
                            start=(kt == 0), stop=(kt == 3))
                ob = op.tile([C, 1024], f32, tag="ob")
                nc.scalar.copy(out=ob[:, :], in_=pob[:, :])
                nc.sync.dma_start(
                    out=OUT[bo // 2, boff:boff + C, c2 * 1024:(c2 + 1) * 1024],
                    in_=ob[:, :])

            for b in range(B):      # batch-major: out(b) unblocks after 4 tiles
                for i in range(4):  # 128-token blocks of this core's slice
                    ksl = kst[:, b, i * 128:(i + 1) * 128]       # [128,128] hi;lo
                    ksh = kst[0:C, b, i * 128:(i + 1) * 128]     # [64,128] hi only
                    sim = simp.tile([128, T], f32, tag="sim")
                    for c2 in range(4):
                        ps = pp.tile([128, 1024], f32, tag="ps")
                        for q in range(2):
                            ch = c2 * 2 + q
                            rh = qhd[b][:, ch * 512:(ch + 1) * 512]
                            rl = qlt[b][:, ch * 512:(ch + 1) * 512]
                            po_s = ps[:, q * 512:(q + 1) * 512]
                            # hi.hi + lo.hi (stacked contraction 128), then hi.lo
                            nc.tensor.matmul(out=po_s, lhsT=ksl, rhs=rh,
                                             start=True, stop=False)
                            nc.tensor.matmul(out=po_s, lhsT=ksh, rhs=rl,
                                             start=False, stop=True)
                        if b == 0 and i == 0 and c2 % 2 == 1:
                            # first tile: split copies across ACT and DVE to
                            # shorten the startup chain to the first Max
                            nc.vector.tensor_copy(
                                out=sim[:, c2 * 1024:(c2 + 1) * 1024],
                                in_=ps[:, :])
                        else:
                            nc.scalar.copy(out=sim[:, c2 * 1024:(c2 + 1) * 1024],
                                           in_=ps[:, :])

                    # previous batch's out chunk c2=i runs here: after this
                    # tile's sim matmuls, overlapping its topk
                    if b > 0:
                        emit_out_chunk(b - 1, i)
                    if b == 3 and i == 3:
                        # batch 3's kt 0..2 accumulate during this tile's topk;
                        # only the kt=3 block remains for the tail
                        for c2 in range(4):
                            for kt in range(3):
                                for q in range(2):
                                    ch = c2 * 2 + q
                                    nc.tensor.matmul(
                                        out=pob3[c2][:, q * 512:(q + 1) * 512],
                                        lhsT=ytp[(3, kt)][:, :],
                                        rhs=wot[kt][:, ch * 512:(ch + 1) * 512],
                                        start=(kt == 0), stop=False)
                    m8 = sp.tile([128, 8], f32, tag="m8")
                    i8 = sp.tile([128, 8], u32, tag="i8")
                    nc.vector.max(out=m8[:, :], in_=sim[:, :])
                    nc.vector.max_index(out=i8[:, :], in_max=m8[:, :], in_values=sim[:, :])
                    gth = sp.tile([128, K, C], f16, tag="gth")
                    for k in range(K):
                        nc.gpsimd.indirect_dma_start(
                            out=gth[:, k, :], out_offset=None,
                            in_=UT[b][k][:, :],
                            in_offset=bass.IndirectOffsetOnAxis(ap=i8[:, k:k + 1], axis=0))
                    t0 = sp.tile([128, C], f16, tag="t0")
                    t1 = sp.tile([128, C], f16, tag="t1")
                    nc.gpsimd.tensor_add(out=t0[:, :], in0=gth[:, 0, :], in1=gth[:, 1, :])
                    nc.gpsimd.tensor_add(out=t1[:, :], in0=gth[:, 2, :], in1=gth[:, 3, :])
                    nc.gpsimd.tensor_add(out=ytp[(b, i)][:, :], in0=t0[:, :],
                                         in1=t1[:, :])

            # tail: batch 3's final kt block; copies split across ACT and DVE
            for c2 in range(4):
                pob = pob3[c2]
                for q in range(2):
                    ch = c2 * 2 + q
                    nc.tensor.matmul(
                        out=pob[:, q * 512:(q + 1) * 512],
                        lhsT=ytp[(3, 3)][:, :],
                        rhs=wot[3][:, ch * 512:(ch + 1) * 512],
                        start=False, stop=True)
                ob = op.tile([C, 1024], f32, tag=f"ob3_{c2}", name=f"ob3{c2}")
                if c2 % 2 == 0:
                    nc.scalar.copy(out=ob[:, :], in_=pob[:, :])
                else:
                    nc.vector.tensor_copy(out=ob[:, :], in_=pob[:, :])
                nc.sync.dma_start(
                    out=OUT[1, C:2 * C, c2 * 1024:(c2 + 1) * 1024],
                    in_=ob[:, :])
    nc.compile()
    return nc


def _split16(a):
    h = a.astype(np.float16)
    l = (a - h.astype(np.float32)).astype(np.float16)
    return h, l


def kernel(x, Wq, Wk, Wv, Wo, conv_w, conv_b):
    x = np.asarray(x, np.float32)
    Wq = np.asarray(Wq, np.float32); Wk = np.asarray(Wk, np.float32)
    Wv = np.asarray(Wv, np.float32); Wo = np.asarray(Wo, np.float32)
    conv_w = np.asarray(conv_w, np.float32); conv_b = np.asarray(conv_b, np.float32)

    if "l1" not in _cache:
        _cache["l1"] = _build_l1()
    if "l2" not in _cache:
        _cache["l2"] = _build_l2()

    def _sw(a):
        # [T, W] -> [128, T//128, W] with [p, kt, w] = a[kt*128+p, w]
        return np.ascontiguousarray(a.reshape(T // 128, 128, -1).transpose(1, 0, 2))

    xT = np.ascontiguousarray(x.transpose(2, 0, 1).reshape(T, BC))  # [t, b*64+c]
    xh, xl = _split16(xT)
    xh, xl = _sw(xh), _sw(xl)
    WqT, WkT = Wq.T, Wk.T
    WvT16 = np.ascontiguousarray(Wv.T).astype(np.float16)
    cw1 = np.ascontiguousarray(conv_w.transpose(1, 2, 0).reshape(C, K * C)).astype(np.float16)
    cw = np.concatenate([cw1, cw1], axis=0)
    # cw[ci, k*64+co] = conv_w[co, ci, k]

    in_maps = []
    for j in range(NCORES):
        sl = slice(j * OS, (j + 1) * OS)
        wqh, wql = _split16(np.ascontiguousarray(WqT[:, sl]))
        wkh, wkl = _split16(np.ascontiguousarray(WkT[:, sl]))
        in_maps.append({"xh": xh, "xl": xl,
                        "wqh": _sw(wqh), "wql": _sw(wql),
                        "wkh": _sw(wkh), "wkl": _sw(wkl),
                        "wv": _sw(np.ascontiguousarray(WvT16[:, sl])), "cw": cw})
    global _last_exec_ns
    _last_exec_ns = []
    r1 = bass_utils.run_bass_kernel_spmd(_cache["l1"], in_maps, core_ids=list(range(NCORES)))
    _last_exec_ns.append(r1.exec_time_ns)

    qh1 = np.concatenate([r1.results[j]["qh_o"] for j in range(NCORES)], axis=2)  # [B,C,T]
    qh = np.ascontiguousarray(np.concatenate([qh1, qh1], axis=1))  # [B,128,T] dup
    ql = np.ascontiguousarray(np.concatenate(
        [r1.results[j]["ql_o"] for j in range(NCORES)], axis=2))
    ut = {}
    for b in range(B):
        for k in range(K):
            ut[(b, k)] = np.ascontiguousarray(np.concatenate(
                [r1.results[j]["u_o"][b, k].reshape(OS, C) for j in range(NCORES)], axis=0))

    in_maps2 = []
    for j in range(NCORES):
        kst_bj = np.concatenate(
            [r1.results[j]["ksh_o"], r1.results[j]["ksl_o"]], axis=1)  # [B,128,OS]
        kst_j = np.ascontiguousarray(
            kst_bj.transpose(1, 0, 2).reshape(128, B * OS))
        m = {"qh": qh, "ql": ql, "kst": kst_j,
             "wot": np.ascontiguousarray(Wo.T[j * OS:(j + 1) * OS, :]).astype(np.float16)}
        for b in range(B):
            for k in range(K):
                m[f"ut{b}_{k}"] = ut[(b, k)]
        in_maps2.append(m)
    r2 = bass_utils.run_bass_kernel_spmd(_cache["l2"], in_maps2, core_ids=list(range(NCORES)))
    _last_exec_ns.append(r2.exec_time_ns)

    out = np.zeros((B, C, T), np.float32)
    for j in range(NCORES):
        oo = r2.results[j]["out_o"]  # [2, 128, T]
        for b in range(B):
            out[b] += oo[b // 2, (b % 2) * C:(b % 2) * C + C, :]
    bias = conv_b[:, None] * Wo.sum(axis=1)[None, :]  # [64, 4096]
    out += bias[None, :, :]
    return out


# revision 39
# speedup vs baseline: 1.0679x; 1.0135x over previous
import sys
for p in ('/opt/trn_rl_repo', '/opt/pypackages'):
    if p not in sys.path:
        sys.path.insert(0, p)
import numpy as np
from concourse import bass, bacc, tile, mybir
from concourse import bass_utils

B, C, T, K = 4, 64, 4096, 4
NCORES = 8
OS = T // NCORES          # 512: per-core token slice (phase-1 o-slice == phase-2 t-slice)
BC = B * C                # 256
f32 = mybir.dt.float32
f16 = mybir.dt.float16
u32 = mybir.dt.uint32

_cache = {}
_last_exec_ns = []


def _build_l1():
    """Phase 1, SPMD core j: q/k/v projections for token slice j via fp16-split
    matmuls (one Ldweights per x tile serves all 7 uses), conv-folded u tables,
    then emit pre-split outputs: qn hi/lo f16, k hi/lo stacked f16."""
    nc = bacc.Bacc("TRN2", target_bir_lowering=False, debug=False, num_devices=NCORES)
    NKT = T // 128            # 32 contraction tiles
    HC = 4                    # kt per streaming chunk
    NCH = NKT // HC           # 4 chunks
    XH = nc.dram_tensor("xh", [128, NKT, BC], f16, kind="ExternalInput").ap()
    XL = nc.dram_tensor("xl", [128, NKT, BC], f16, kind="ExternalInput").ap()
    WQH = nc.dram_tensor("wqh", [128, NKT, OS], f16, kind="ExternalInput").ap()
    WQL = nc.dram_tensor("wql", [128, NKT, OS], f16, kind="ExternalInput").ap()
    WKH = nc.dram_tensor("wkh", [128, NKT, OS], f16, kind="ExternalInput").ap()
    WKL = nc.dram_tensor("wkl", [128, NKT, OS], f16, kind="ExternalInput").ap()
    WV = nc.dram_tensor("wv", [128, NKT, OS], f16, kind="ExternalInput").ap()
    CW = nc.dram_tensor("cw", [2 * C, K * C], f16, kind="ExternalInput").ap()
    QHO = nc.dram_tensor("qh_o", [B, C, OS], f16, kind="ExternalOutput").ap()
    QLO = nc.dram_tensor("ql_o", [B, C, OS], f16, kind="ExternalOutput").ap()
    KSH = nc.dram_tensor("ksh_o", [B, C, OS], f16, kind="ExternalOutput").ap()
    KSL = nc.dram_tensor("ksl_o", [B, C, OS], f16, kind="ExternalOutput").ap()
    UO = nc.dram_tensor("u_o", [B, K, 4, 128, C], f16, kind="ExternalOutput").ap()

    with tile.TileContext(nc) as tc:
        with tc.tile_pool(name="xp", bufs=2) as xp, \
             tc.tile_pool(name="wp", bufs=2) as wp, \
             tc.tile_pool(name="sp", bufs=1) as sp, \
             tc.tile_pool(name="up", bufs=4) as up, \
             tc.tile_pool(name="tp", bufs=2) as tp, \
             tc.tile_pool(name="cp", bufs=1) as cp, \
             tc.tile_pool(name="ppq", bufs=1, space="PSUM") as ppq, \
             tc.tile_pool(name="ppk", bufs=1, space="PSUM") as ppk, \
             tc.tile_pool(name="ppv", bufs=1, space="PSUM") as ppv, \
             tc.tile_pool(name="pu", bufs=1, space="PSUM") as pu:
            cw = cp.tile([2 * C, K * C], f16, tag="cw")
            nc.sync.dma_start(out=cw[:, :], in_=CW[:, :])
            ones_r = cp.tile([128, 1], f32, tag="ones_r")   # reduce lhsT
            nc.vector.memset(ones_r[:, :], 1.0)
            ones_b = cp.tile([1, C], f32, tag="ones_b")     # broadcast lhsT
            nc.vector.memset(ones_b[:, :], 1.0)

            gate = cp.tile([1, 1], f16, tag="gate")
            qacc = [ppq.tile([128, OS], f32, tag=f"q{mt}", name=f"qacc{mt}")
                    for mt in range(2)]
            kacc = [ppk.tile([128, OS], f32, tag=f"k{mt}", name=f"kacc{mt}")
                    for mt in range(2)]
            vacc = [ppv.tile([128, OS], f32, tag=f"v{mt}", name=f"vacc{mt}")
                    for mt in range(2)]

            for ch in range(NCH):
                ksl = slice(ch * HC, (ch + 1) * HC)
                xh = xp.tile([128, HC, BC], f16, tag="xh")
                xl = xp.tile([128, HC, BC], f16, tag="xl")
                nc.sync.dma_start(out=xh[:, :, :], in_=XH[:, ksl, :])
                nc.sync.dma_start(out=xl[:, :, :], in_=XL[:, ksl, :])
                wqh = wp.tile([128, HC, OS], f16, tag="wqh")
                wql = wp.tile([128, HC, OS], f16, tag="wql")
                wkh = wp.tile([128, HC, OS], f16, tag="wkh")
                wkl = wp.tile([128, HC, OS], f16, tag="wkl")
                wv = wp.tile([128, HC, OS], f16, tag="wv")
                nc.scalar.dma_start(out=wqh[:, :, :], in_=WQH[:, ksl, :])
                nc.gpsimd.dma_start(out=wkh[:, :, :], in_=WKH[:, ksl, :])
                nc.scalar.dma_start(out=wql[:, :, :], in_=WQL[:, ksl, :])
                nc.gpsimd.dma_start(out=wkl[:, :, :], in_=WKL[:, ksl, :])
                nc.gpsimd.dma_start(out=wv[:, :, :], in_=WV[:, ksl, :])
                for t in range(HC):
                    kt = ch * HC + t
                    first = (kt == 0)
                    last = (kt == NKT - 1)
                    for mt in range(2):
                        lh = xh[:, t, mt * 128:(mt + 1) * 128]
                        ll = xl[:, t, mt * 128:(mt + 1) * 128]
                        # one Ldweights of lh serves 5 matmuls, ll serves 2
                        nc.tensor.matmul(out=qacc[mt][:, :], lhsT=lh, rhs=wqh[:, t, :],
                                         start=first, stop=False)
                        nc.tensor.matmul(out=kacc[mt][:, :], lhsT=lh, rhs=wkh[:, t, :],
                                         start=first, stop=False)
                        nc.tensor.matmul(out=vacc[mt][:, :], lhsT=lh, rhs=wv[:, t, :],
                                         start=first, stop=last)
                        nc.tensor.matmul(out=qacc[mt][:, :], lhsT=lh, rhs=wql[:, t, :],
                                         start=False, stop=False)
                        nc.tensor.matmul(out=kacc[mt][:, :], lhsT=lh, rhs=wkl[:, t, :],
                                         start=False, stop=False)
                        nc.tensor.matmul(out=qacc[mt][:, :], lhsT=ll, rhs=wqh[:, t, :],
                                         start=False, stop=last)
                        nc.tensor.matmul(out=kacc[mt][:, :], lhsT=ll, rhs=wkh[:, t, :],
                                         start=False, stop=last)

            # ---- V to f16 ----
            vsb = {}
            for mt in range(2):
                v16 = sp.tile([128, OS], f16, tag=f"vsb{mt}")
                nc.scalar.copy(out=v16[:, :], in_=vacc[mt][:, :])
                vsb[mt] = v16
            pub = [pu.tile([1, OS], f32, tag=f"pub{j}", name=f"pub{j}")
                   for j in range(2)]
            # ---- k: split into f16 hi/lo ----
            for mt in range(2):
                ks = tp.tile([128, OS], f32, tag=f"ksb{mt}")
                nc.scalar.copy(out=ks[:, :], in_=kacc[mt][:, :])
                kh = tp.tile([128, OS], f16, tag="kh")
                nc.scalar.copy(out=kh[:, :], in_=ks[:, :])
                kl = tp.tile([128, OS], f16, tag="kl")
                nc.vector.tensor_sub(out=kl[:, :], in0=ks[:, :], in1=kh[:, :])
                ksh2 = KSH[2 * mt:2 * mt + 2].rearrange("b c o -> (b c) o")
                ksl2 = KSL[2 * mt:2 * mt + 2].rearrange("b c o -> (b c) o")
                nc.gpsimd.dma_start(out=ksh2[:, :], in_=kh[:, :])
                nc.gpsimd.dma_start(out=ksl2[:, :], in_=kl[:, :])
            # ---- qn norms: squares + row-sums + sqrt + recip, pipelined ----
            sq, qn = {}, {}
            for mt in range(2):
                sq[mt] = tp.tile([128, OS], f32, tag=f"sq{mt}", name=f"sq{mt}")
                nc.scalar.square(out=sq[mt][:, :], in_=qacc[mt][:, :])
                qn[mt] = tp.tile([128, OS], f32, tag=f"qn{mt}", name=f"qn{mt}")
            nrm, rcp = {}, {}
            for b in range(B):
                mt, off = b // 2, (b % 2) * C
                nc.tensor.matmul(out=pub[b % 2][:, :], lhsT=ones_r[off:off + C, :],
                                 rhs=sq[mt][off:off + C, :], start=True, stop=True)
                nrm[b] = sp.tile([1, OS], f32, tag=f"nrm{b}", name=f"nrm{b}")
                nc.scalar.sqrt(out=nrm[b][:, :], in_=pub[b % 2][:, :])
            for b in range(B):
                rcp[b] = sp.tile([1, OS], f32, tag=f"rcp{b}", name=f"rcp{b}")
                nc.vector.reciprocal(out=rcp[b][:, :], in_=nrm[b][:, :])
            # ---- u tables (PE + ACT copies interleave with the norm tail) ----
            for b in range(B):
                off = (b % 2) * C
                vt = vsb[b // 2][off:off + C, :]  # [64, 512] f16
                ust = up.tile([128, K, 4, C], f16, tag="ust")
                for k in range(K):
                    # two k-tables per vacc tile -> one wide copy per pair
                    pacc = vacc[k // 2][:, (k % 2) * 4 * C:(k % 2 + 1) * 4 * C]
                    for m in range(4):
                        nc.tensor.matmul(out=pacc[:, m * C:(m + 1) * C],
                                         lhsT=vt[:, m * 128:(m + 1) * 128],
                                         rhs=cw[off:off + C, k * C:(k + 1) * C],
                                         start=True, stop=True)
                    if k % 2 == 1:
                        if k == 1:
                            nc.scalar.copy(out=ust[:, k - 1:k + 1, :, :],
                                           in_=vacc[k // 2][:, :])
                        else:
                            nc.vector.tensor_copy(out=ust[:, k - 1:k + 1, :, :],
                                                  in_=vacc[k // 2][:, :])
                nc.sync.dma_start(
                    out=UO[b].rearrange("k m p c -> p k m c")[:, :, :, :],
                    in_=ust[:, :, :, :])
            # ---- qn broadcast-scale + split hi/lo ----
            for b in range(B):
                mt, off = b // 2, (b % 2) * C
                nc.tensor.matmul(out=kacc[mt][off:off + C, :], lhsT=ones_b[:, :],
                                 rhs=rcp[b][:, :], start=True, stop=True)
            for mt in range(2):
                bc = tp.tile([128, OS], f32, tag=f"bc{mt}", name=f"bc{mt}")
                nc.scalar.copy(out=bc[:, :], in_=kacc[mt][:, :])
                nc.vector.tensor_mul(out=qn[mt][:, :], in0=qacc[mt][:, :],
                                     in1=bc[:, :])
            for mt in range(2):
                qh = tp.tile([128, OS], f16, tag="qh")
                nc.scalar.copy(out=qh[:, :], in_=qn[mt][:, :])
                qlo = tp.tile([128, OS], f16, tag="qlo")
                nc.vector.tensor_sub(out=qlo[:, :], in0=qn[mt][:, :], in1=qh[:, :])
                qho2 = QHO[2 * mt:2 * mt + 2].rearrange("b c o -> (b c) o")
                qlo2 = QLO[2 * mt:2 * mt + 2].rearrange("b c o -> (b c) o")
                nc.scalar.dma_start(out=qho2[:, :], in_=qh[:, :])
                nc.scalar.dma_start(out=qlo2[:, :], in_=qlo[:, :])
    nc.compile()
    return nc


def _build_l2():
    """Phase 2, SPMD core j: rows t in [j*512,(j+1)*512) for all batches.
    sim = k_t . qn_s via stacked [hi;lo] 2-matmul fp16 split, exact top-4 via
    max/max_index, gather-sum of u tables. Batch-major tile order so each
    batch's out matmul spreads through the run; batch 3's accumulates
    cross-phase in persistent psum so only the stop-chunk lands in the tail."""
    nc = bacc.Bacc("TRN2", target_bir_lowering=False, debug=False, num_devices=NCORES)
    QH = nc.dram_tensor("qh", [B, 128, T], f16, kind="ExternalInput").ap()
    QL = nc.dram_tensor("ql", [B, C, T], f16, kind="ExternalInput").ap()
    KST = nc.dram_tensor("kst", [128, B * OS], f16, kind="ExternalInput").ap()
    WOT = nc.dram_tensor("wot", [OS, T], f16, kind="ExternalInput").ap()
    UT = [[nc.dram_tensor(f"ut{b}_{k}", [T, C], f16, kind="ExternalInput").ap()
           for k in range(K)] for b in range(B)]
    OUT = nc.dram_tensor("out_o", [2, 128, T], f32, kind="ExternalOutput").ap()

    with tile.TileContext(nc) as tc:
        with tc.tile_pool(name="qp", bufs=1) as qp, \
             tc.tile_pool(name="wp", bufs=1) as wp, \
             tc.tile_pool(name="sp", bufs=2) as sp, \
             tc.tile_pool(name="simp", bufs=3) as simp, \
             tc.tile_pool(name="yp", bufs=1) as yp, \
             tc.tile_pool(name="op", bufs=2) as op, \
             tc.tile_pool(name="pp", bufs=2, space="PSUM") as pp, \
             tc.tile_pool(name="po", bufs=1, space="PSUM") as po:
            kst_flat = qp.tile([128, B * OS], f16, tag="kst")
            kst = kst_flat.rearrange("p (b o) -> p b o", b=B)
            qhd, qlt = {}, {}
            for b in range(B):
                qhd[b] = qp.tile([128, T], f16, tag=f"qhd{b}", name=f"qhd{b}")
                qlt[b] = qp.tile([C, T], f16, tag=f"qlt{b}", name=f"qlt{b}")
            # all loads on the SP queue in need-order: ACT must stay clear for
            # sim copies, gpsimd for gathers
            nc.scalar.dma_start(out=kst_flat[:, :], in_=KST[:, :])
            nc.sync.dma_start(out=qhd[0][:, 0:2048], in_=QH[0][:, 0:2048])
            nc.sync.dma_start(out=qlt[0][:, 0:2048], in_=QL[0][:, 0:2048])
            nc.sync.dma_start(out=qhd[0][:, 2048:T], in_=QH[0][:, 2048:T])
            nc.sync.dma_start(out=qlt[0][:, 2048:T], in_=QL[0][:, 2048:T])
            wot = {}
            for kt in range(4):
                wot[kt] = wp.tile([128, T], f16, tag=f"wot{kt}", name=f"wot{kt}")
            for b in range(1, B):
                nc.sync.dma_start(out=qhd[b][:, :], in_=QH[b])
                nc.sync.dma_start(out=qlt[b][:, :], in_=QL[b])
            for kt in range(4):
                nc.sync.dma_start(out=wot[kt][:, :],
                                  in_=WOT[kt * 128:(kt + 1) * 128, :])

            ytp = {}  # (b, kt) -> [128, C] f16 lhsT tiles for the out matmul
            for b in range(B):
                for kt in range(4):
                    ytp[(b, kt)] = yp.tile([128, C], f16, tag=f"yt{b}{kt}",
                                           name=f"ytp{b}{kt}")
            pot = [po.tile([128, 1024], f32, tag=f"pot{j}", name=f"pot{j}")
                   for j in range(2)]
            pob3 = [pot[c2 // 2][(c2 % 2) * C:(c2 % 2) * C + C, :]
                    for c2 in range(4)]

            def emit_out_chunk(bo, c2):
                # one 1024-col chunk of batch bo's out; psum from shared pool
                boff = (bo % 2) * C
                pob = pp.tile([128, 1024], f32, tag="ps", name="pob")[0:C, :]
                for kt in range(4):
                    for q in range(2):
                        ch = c2 * 2 + q
                        nc.tensor.matmul(
                            out=pob[:, q * 512:(q + 1) * 512],
                            lhsT=ytp[(bo, kt)][:, :],
                            rhs=wot[kt][:, ch * 512:(ch + 1) * 512],
                            start=(kt == 0), stop=(kt == 3))
                ob = op.tile([C, 1024], f32, tag="ob")
                nc.scalar.copy(out=ob[:, :], in_=pob[:, :])
                nc.sync.dma_start(
                    out=OUT[bo // 2, boff:boff + C, c2 * 1024:(c2 + 1) * 1024],
                    in_=ob[:, :])

            for b in range(B):      # batch-major: out(b) unblocks after 4 tiles
                for i in range(4):  # 128-token blocks of this core's slice
                    ksl = kst[:, b, i * 128:(i + 1) * 128]       # [128,128] hi;lo
                    ksh = kst[0:C, b, i * 128:(i + 1) * 128]     # [64,128] hi only
                    sim = simp.tile([128, T], f32, tag="sim")
                    for c2 in range(4):
                        ps = pp.tile([128, 1024], f32, tag="ps")
                        for q in range(2):
                            ch = c2 * 2 + q
                            rh = qhd[b][:, ch * 512:(ch + 1) * 512]
                            rl = qlt[b][:, ch * 512:(ch + 1) * 512]
                            po_s = ps[:, q * 512:(q + 1) * 512]
                            # hi.hi + lo.hi (stacked contraction 128), then hi.lo
                            nc.tensor.matmul(out=po_s, lhsT=ksl, rhs=rh,
                                             start=True, stop=False)
                            nc.tensor.matmul(out=po_s, lhsT=ksh, rhs=rl,
                                             start=False, stop=True)
                        if b == 0 and i == 0 and c2 % 2 == 1:
                            # first tile: split copies across ACT and DVE to
                            # shorten the startup chain to the first Max
                            nc.vector.tensor_copy(
                                out=sim[:, c2 * 1024:(c2 + 1) * 1024],
                                in_=ps[:, :])
                        else:
                            nc.scalar.copy(out=sim[:, c2 * 1024:(c2 + 1) * 1024],
                                           in_=ps[:, :])

                    # previous batch's out chunk c2=i runs here: after this
                    # tile's sim matmuls, overlapping its topk
                    if b > 0:
                        emit_out_chunk(b - 1, i)
                    if b == 3 and i == 3:
                        # batch 3's kt 0..2 accumulate during this tile's topk;
                        # only the kt=3 block remains for the tail
                        for c2 in range(4):
                            for kt in range(3):
                                for q in range(2):
                                    ch = c2 * 2 + q
                                    nc.tensor.matmul(
                                        out=pob3[c2][:, q * 512:(q + 1) * 512],
                                        lhsT=ytp[(3, kt)][:, :],
                                        rhs=wot[kt][:, ch * 512:(ch + 1) * 512],
                                        start=(kt == 0), stop=False)
                    m8 = sp.tile([128, 8], f32, tag="m8")
                    i8 = sp.tile([128, 8], u32, tag="i8")
                    nc.vector.max(out=m8[:, :], in_=sim[:, :])
                    nc.vector.max_index(out=i8[:, :], in_max=m8[:, :], in_values=sim[:, :])
                    gth = sp.tile([128, K, C], f16, tag="gth")
                    for k in range(K):
                        nc.gpsimd.indirect_dma_start(
                            out=gth[:, k, :], out_offset=None,
                            in_=UT[b][k][:, :],
                            in_offset=bass.IndirectOffsetOnAxis(ap=i8[:, k:k + 1], axis=0))
                    t0 = sp.tile([128, C], f16, tag="t0")
                    t1 = sp.tile([128, C], f16, tag="t1")
                    nc.gpsimd.tensor_add(out=t0[:, :], in0=gth[:, 0, :], in1=gth[:, 1, :])
                    nc.gpsimd.tensor_add(out=t1[:, :], in0=gth[:, 2, :], in1=gth[:, 3, :])
                    nc.gpsimd.tensor_add(out=ytp[(b, i)][:, :], in0=t0[:, :],
                                         in1=t1[:, :])

            # tail: batch 3's final kt block; copies split across ACT and DVE
            for c2 in range(4):
                pob = pob3[c2]
                for q in range(2):
                    ch = c2 * 2 + q
                    nc.tensor.matmul(
                        out=pob[:, q * 512:(q + 1) * 512],
                        lhsT=ytp[(3, 3)][:, :],
                        rhs=wot[3][:, ch * 512:(ch + 1) * 512],
                        start=False, stop=True)
                ob = op.tile([C, 1024], f32, tag=f"ob3_{c2}", name=f"ob3{c2}")
                if c2 % 2 == 0:
                    nc.scalar.copy(out=ob[:, :], in_=pob[:, :])
                else:
                    nc.vector.tensor_copy(out=ob[:, :], in_=pob[:, :])
                nc.sync.dma_start(
                    out=OUT[1, C:2 * C, c2 * 1024:(c2 + 1) * 1024],
                    in_=ob[:, :])
    nc.compile()
    return nc


def _split16(a):
    h = a.astype(np.float16)
    l = (a - h.astype(np.float32)).astype(np.float16)
    return h, l


def kernel(x, Wq, Wk, Wv, Wo, conv_w, conv_b):
    x = np.asarray(x, np.float32)
    Wq = np.asarray(Wq, np.float32); Wk = np.asarray(Wk, np.float32)
    Wv = np.asarray(Wv, np.float32); Wo = np.asarray(Wo, np.float32)
    conv_w = np.asarray(conv_w, np.float32); conv_b = np.asarray(conv_b, np.float32)

    if "l1" not in _cache:
        _cache["l1"] = _build_l1()
    if "l2" not in _cache:
        _cache["l2"] = _build_l2()

    def _sw(a):
        # [T, W] -> [128, T//128, W] with [p, kt, w] = a[kt*128+p, w]
        return np.ascontiguousarray(a.reshape(T // 128, 128, -1).transpose(1, 0, 2))

    xT = np.ascontiguousarray(x.transpose(2, 0, 1).reshape(T, BC))  # [t, b*64+c]
    xh, xl = _split16(xT)
    xh, xl = _sw(xh), _sw(xl)
    WqT, WkT = Wq.T, Wk.T
    WvT16 = np.ascontiguousarray(Wv.T).astype(np.float16)
    cw1 = np.ascontiguousarray(conv_w.transpose(1, 2, 0).reshape(C, K * C)).astype(np.float16)
    cw = np.concatenate([cw1, cw1], axis=0)
    # cw[ci, k*64+co] = conv_w[co, ci, k]

    in_maps = []
    for j in range(NCORES):
        sl = slice(j * OS, (j + 1) * OS)
        wqh, wql = _split16(np.ascontiguousarray(WqT[:, sl]))
        wkh, wkl = _split16(np.ascontiguousarray(WkT[:, sl]))
        in_maps.append({"xh": xh, "xl": xl,
                        "wqh": _sw(wqh), "wql": _sw(wql),
                        "wkh": _sw(wkh), "wkl": _sw(wkl),
                        "wv": _sw(np.ascontiguousarray(WvT16[:, sl])), "cw": cw})
    global _last_exec_ns
    _last_exec_ns = []
    r1 = bass_utils.run_bass_kernel_spmd(_cache["l1"], in_maps, core_ids=list(range(NCORES)))
    _last_exec_ns.append(r1.exec_time_ns)

    qh1 = np.concatenate([r1.results[j]["qh_o"] for j in range(NCORES)], axis=2)  # [B,C,T]
    qh = np.ascontiguousarray(np.concatenate([qh1, qh1], axis=1))  # [B,128,T] dup
    ql = np.ascontiguousarray(np.concatenate(
        [r1.results[j]["ql_o"] for j in range(NCORES)], axis=2))
    ut = {}
    for b in range(B):
        for k in range(K):
            ut[(b, k)] = np.ascontiguousarray(np.concatenate(
                [r1.results[j]["u_o"][b, k].reshape(OS, C) for j in range(NCORES)], axis=0))

    in_maps2 = []
    for j in range(NCORES):
        kst_bj = np.concatenate(
            [r1.results[j]["ksh_o"], r1.results[j]["ksl_o"]], axis=1)  # [B,128,OS]
        kst_j = np.ascontiguousarray(
            kst_bj.transpose(1, 0, 2).reshape(128, B * OS))
        m = {"qh": qh, "ql": ql, "kst": kst_j,
             "wot": np.ascontiguousarray(Wo.T[j * OS:(j + 1) * OS, :]).astype(np.float16)}
        for b in range(B):
            for k in range(K):
                m[f"ut{b}_{k}"] = ut[(b, k)]
        in_maps2.append(m)
    r2 = bass_utils.run_bass_kernel_spmd(_cache["l2"], in_maps2, core_ids=list(range(NCORES)))
    _last_exec_ns.append(r2.exec_time_ns)

    out = np.zeros((B, C, T), np.float32)
    for j in range(NCORES):
        oo = r2.results[j]["out_o"]  # [2, 128, T]
        for b in range(B):
            out[b] += oo[b // 2, (b % 2) * C:(b % 2) * C + C, :]
    bias = conv_b[:, None] * Wo.sum(axis=1)[None, :]  # [64, 4096]
    out += bias[None, :, :]
    return out
